# revision 25
# baseline (speedup 1.0000x reference)
"""Trainium2 Bass kernel for GQA causal attention (B=2, T=2048, H=16, KV=4, D=128).

Sharding: 8 cores = (batch b in {0,1}) x (kv-group g in {0..3}).
Each core computes 4 Q heads + 1 KV head for one batch:
  Q/K/V projections (column-parallel weights), RoPE, causal attention,
  row-parallel Wo partials, ReduceScatter within the 4-core batch group.

Token blocks are processed in 4 contiguous groups of decreasing size
[7,5,3,1] blocks; emission interleaves proj(g) -> attn(g) -> RS(g) so the
PE stream never waits on a phase barrier and the last ReduceScatter is
tiny (0.5 MB).  Each core returns its summed output rows; the host
reassembles.
"""

import math

import numpy as np

import concourse.mybir as mybir
import concourse.tile as tile
from concourse import bacc
from concourse.bass_utils import run_bass_kernel_spmd
from concourse.masks import make_identity

F32 = mybir.dt.float32
BF16 = mybir.dt.bfloat16
EXP = mybir.ActivationFunctionType.Exp
MULT = mybir.AluOpType.mult

B, T, C = 2, 2048, 2048
H, KH, D = 16, 4, 128
R = H // KH  # q heads per kv group (4)
N_CORES = 8
TI = T // 128  # 16 token blocks
EO = C // 128  # 16 embedding chunks
SCALE = 1.0 / math.sqrt(D)

# decreasing group sizes: later query blocks attend more keys, so later
# groups get fewer blocks -> roughly equal work + a tiny final RS.
GBS = [7, 5, 3, 1]  # blocks per group
LOB = [0, 7, 12, 15]  # first block of each group
NG = len(GBS)
ROWS_G = [gb * 32 for gb in GBS]  # RS output rows per core per group

_CACHE = {}


def _build_program():
    nc = bacc.Bacc(
        "TRN2", target_bir_lowering=False, debug=False, num_devices=N_CORES
    )

    x_d = nc.dram_tensor("x", [T, C], F32, kind="ExternalInput").ap()
    cos_d = nc.dram_tensor("cos", [T, D], F32, kind="ExternalInput").ap()
    sin_d = nc.dram_tensor("sin", [T, D], F32, kind="ExternalInput").ap()
    wq_d = nc.dram_tensor("wq", [C, R * D], F32, kind="ExternalInput").ap()
    wk_d = nc.dram_tensor("wk", [C, D], F32, kind="ExternalInput").ap()
    wv_d = nc.dram_tensor("wv", [C, D], F32, kind="ExternalInput").ap()
    wo_d = nc.dram_tensor("wo", [R * D, C], F32, kind="ExternalInput").ap()
    out_d = nc.dram_tensor("out", [512, C], F32, kind="ExternalOutput").ap()

    with tile.TileContext(nc) as tc:
        _kernel_body(tc, x_d, cos_d, sin_d, wq_d, wk_d, wv_d, wo_d, out_d)

    nc.compile()
    return nc


def _kernel_body(tc, x_d, cos_d, sin_d, wq_d, wk_d, wv_d, wo_d, out_d):
    nc = tc.nc

    # ---- pools (flat: no nested scopes, so no recycling barriers) ----
    consts = tc.alloc_tile_pool(name="consts", bufs=1)
    wts = tc.alloc_tile_pool(name="wts", bufs=1)
    wo_pool = tc.alloc_tile_pool(name="wo", bufs=1)
    projout = tc.alloc_tile_pool(name="projout", bufs=1)
    wstage = tc.alloc_tile_pool(name="wstage", bufs=2)
    xfpool = tc.alloc_tile_pool(name="xfpool", bufs=3)
    xstage = tc.alloc_tile_pool(name="xstage", bufs=2)
    xtb_pool = tc.alloc_tile_pool(name="xtb", bufs=8)
    rope = tc.alloc_tile_pool(name="rope", bufs=2)
    stp = tc.alloc_tile_pool(name="st", bufs=2)
    ypool = tc.alloc_tile_pool(name="ypool", bufs=3)
    outp = tc.alloc_tile_pool(name="outp", bufs=2)
    post = tc.alloc_tile_pool(name="post", bufs=1)
    dram = tc.alloc_tile_pool(name="dram", bufs=1, space="DRAM")
    projps = tc.alloc_tile_pool(name="projps", bufs=2, space="PSUM")
    tpsum = tc.alloc_tile_pool(name="tpsum", bufs=2, space="PSUM")
    sps = tc.alloc_tile_pool(name="sps", bufs=2, space="PSUM")
    pops = tc.alloc_tile_pool(name="pops", bufs=2, space="PSUM")

    # ---- constants ----
    ut_mask = consts.tile([128, 128], BF16)
    nc.gpsimd.memset(ut_mask, 1.0)
    nc.gpsimd.affine_select(
        out=ut_mask,
        in_=ut_mask,
        compare_op=mybir.AluOpType.is_ge,
        fill=0.0,
        base=0,
        pattern=[[1, 128]],
        channel_multiplier=-1,
    )

    cos_sb = consts.tile([128, TI, D], F32)
    sin_sb = consts.tile([128, TI, D], F32)
    nc.gpsimd.dma_start(cos_sb, cos_d.rearrange("(to ti) d -> ti to d", ti=128))
    nc.gpsimd.dma_start(sin_sb, sin_d.rearrange("(to ti) d -> ti to d", ti=128))

    ident_b = consts.tile([128, 128], BF16)
    make_identity(nc, ident_b)

    # ---- PE warm-up: ~10us of dense matmuls to flip HAM to 8/8 while
    # the first x block + weights stream in ----
    wu = consts.tile([128, 512], BF16)
    nc.vector.memset(wu, 0.01)
    wups = projps.tile([128, 512], F32, tag="ps")
    for i in range(36):
        nc.tensor.matmul(wups, lhsT=ident_b, rhs=wu, start=(i == 0), stop=(i == 35))
    wusink = consts.tile([128, 8], F32)
    nc.scalar.copy(wusink, wups[:, 0:8])

    # ---- persistent proj outputs (per token group) ----
    qt_g = [projout.tile([128, R, GBS[g] * 128], BF16, name=f"qt{g}") for g in range(NG)]
    kt_g = [projout.tile([128, GBS[g] * 128], BF16, name=f"kt{g}") for g in range(NG)]
    yt_g = [projout.tile([128, R, GBS[g] * 128], BF16, name=f"yt{g}") for g in range(NG)]
    v_t = [projout.tile([128, 132], BF16, name=f"v{ti}") for ti in range(TI)]
    for ti in range(TI):
        nc.vector.memset(v_t[ti][:, 128:129], 1.0)

    # ---- weights (cast to bf16); per-chunk tiles so the first matmuls
    # only wait on their own chunk, not the whole weight matrix ----
    wq_t = [wts.tile([128, R * D], BF16, name=f"wq{eo}") for eo in range(EO)]
    wk_t = [wts.tile([128, D], BF16, name=f"wk{eo}") for eo in range(EO)]
    wv_t = [wts.tile([128, D], BF16, name=f"wv{eo}") for eo in range(EO)]
    wo_t = [wo_pool.tile([128, C], BF16, name=f"wo{hh}") for hh in range(R)]
    for eo in range(EO):
        st_q = wstage.tile([128, R * D], F32, tag="wst")
        nc.gpsimd.dma_start(st_q, wq_d[eo * 128 : (eo + 1) * 128, :])
        nc.scalar.copy(wq_t[eo], st_q)
        st_k = wstage.tile([128, D], F32, tag="wst_kv")
        nc.gpsimd.dma_start(st_k, wk_d[eo * 128 : (eo + 1) * 128, :])
        nc.scalar.copy(wk_t[eo], st_k)
        st_v = wstage.tile([128, D], F32, tag="wst_kv")
        nc.gpsimd.dma_start(st_v, wv_d[eo * 128 : (eo + 1) * 128, :])
        nc.scalar.copy(wv_t[eo], st_v)
    for hh in range(R):
        for no in range(4):
            st_o = wstage.tile([128, 512], F32, tag="wst")
            nc.gpsimd.dma_start(
                st_o, wo_d[hh * 128 : (hh + 1) * 128, no * 512 : (no + 1) * 512]
            )
            nc.scalar.copy(wo_t[hh][:, no * 512 : (no + 1) * 512], st_o)

    rs_tiles = []

    for g in range(NG):
        lo, hi = LOB[g] * 128, (LOB[g] + GBS[g]) * 128

        # ---- projection for this group's token blocks ----
        for bi in range(GBS[g]):
            ti = LOB[g] + bi
            tl = bi * 128

            xf = xfpool.tile([128, C], F32, tag="xf")
            for q4 in range(4):
                nc.sync.dma_start(
                    xf[:, q4 * 512 : (q4 + 1) * 512],
                    x_d[ti * 128 : (ti + 1) * 128, q4 * 512 : (q4 + 1) * 512],
                )
            xb = xstage.tile([128, C], BF16, tag="xb")
            nc.vector.tensor_copy(xb, xf)
            xt4 = []
            for e4 in range(4):
                tp = tpsum.tile([128, 4, 128], BF16, tag="tp")
                for k in range(4):
                    eo = e4 * 4 + k
                    nc.tensor.transpose(
                        tp[:, k, :], xb[:, eo * 128 : (eo + 1) * 128], ident_b
                    )
                xt_c = xtb_pool.tile([128, 4, 128], BF16, tag="xt")
                nc.vector.tensor_copy(xt_c, tp)
                xt4.append(xt_c)

            # Q: psum [tok, R*D]
            psq = projps.tile([128, R * D], F32, tag="ps")
            for eo in range(EO):
                nc.tensor.matmul(
                    psq,
                    lhsT=xt4[eo // 4][:, eo % 4, :],
                    rhs=wq_t[eo],
                    start=(eo == 0),
                    stop=(eo == EO - 1),
                )
            psq_v = psq[:, :].rearrange("p (h d) -> p h d", h=R)
            cos_bc = cos_sb[:, ti, None, :].to_broadcast((128, R, D))
            sin_bc = sin_sb[:, ti, None, :].to_broadcast((128, R, D))
            tc_t = rope.tile([128, R, D], BF16, tag="ropeC")
            ts_t = rope.tile([128, R, D], BF16, tag="ropeS")
            nc.vector.tensor_tensor(tc_t, psq_v, cos_bc, MULT)
            nc.vector.tensor_tensor(ts_t, psq_v, sin_bc, MULT)
            qb = rope.tile([128, R, D], BF16, tag="qb")
            nc.vector.tensor_sub(qb[:, :, 0:64], tc_t[:, :, 0:64], ts_t[:, :, 64:128])
            nc.vector.tensor_add(
                qb[:, :, 64:128], tc_t[:, :, 64:128], ts_t[:, :, 0:64]
            )
            qtp = tpsum.tile([128, 4, 128], BF16, tag="tp")
            for h in range(R):
                nc.tensor.transpose(qtp[:, h, :], qb[:, h, :], ident_b)
            nc.vector.tensor_copy(qt_g[g][:, :, tl : tl + 128], qtp)

            # K: psum [tok, D]
            psk_t = projps.tile([128, R * D], F32, tag="ps")
            psk = psk_t[:, 0:D]
            for eo in range(EO):
                nc.tensor.matmul(
                    psk,
                    lhsT=xt4[eo // 4][:, eo % 4, :],
                    rhs=wk_t[eo],
                    start=(eo == 0),
                    stop=(eo == EO - 1),
                )
            tck = rope.tile([128, D], BF16, tag="ropeCk")
            tsk = rope.tile([128, D], BF16, tag="ropeSk")
            nc.vector.tensor_tensor(tck, psk, cos_sb[:, ti, :], MULT)
            nc.vector.tensor_tensor(tsk, psk, sin_sb[:, ti, :], MULT)
            kb_t = rope.tile([128, D], BF16, tag="kb")
            nc.vector.tensor_sub(kb_t[:, 0:64], tck[:, 0:64], tsk[:, 64:128])
            nc.vector.tensor_add(kb_t[:, 64:128], tck[:, 64:128], tsk[:, 0:64])
            tpk = tpsum.tile([128, 4, 128], BF16, tag="tp")
            nc.tensor.transpose(tpk[:, 0, :], kb_t, ident_b)
            nc.vector.tensor_copy(kt_g[g][:, tl : tl + 128], tpk[:, 0, :])

            # V: psum [tok, D] -> v_t[ti][:, 0:128]; col 128 = 1.0
            psv_t = projps.tile([128, R * D], F32, tag="ps")
            psv = psv_t[:, 0:D]
            for eo in range(EO):
                nc.tensor.matmul(
                    psv,
                    lhsT=xt4[eo // 4][:, eo % 4, :],
                    rhs=wv_t[eo],
                    start=(eo == 0),
                    stop=(eo == EO - 1),
                )
            nc.scalar.copy(v_t[ti][:, 0:128], psv)

        # ---- attention for this group (keys 0 .. hi) ----
        nkb = LOB[g] + GBS[g]
        offs = {}
        o = 0
        for kb in range(nkb):
            offs[kb] = o
            o += hi - max(kb * 128, lo)
        st_cols = o

        for h in range(R):
            st_all = stp.tile([128, 6400], BF16, tag="st_all")
            for kb in range(nkb):
                s0 = max(kb * 128, lo)
                w = hi - s0
                kg = next(gg for gg in range(NG) if LOB[gg] <= kb < LOB[gg] + GBS[gg])
                kt_src = kt_g[kg][:, (kb - LOB[kg]) * 128 : (kb - LOB[kg] + 1) * 128]
                for m0 in range(0, w, 512):
                    mw = min(512, w - m0)
                    ps5 = sps.tile([128, 512], F32, tag="ps5")
                    nc.tensor.matmul(
                        ps5[:, 0:mw],
                        lhsT=kt_src,
                        rhs=qt_g[g][:, h, s0 - lo + m0 : s0 - lo + m0 + mw],
                        start=True,
                        stop=True,
                    )
                    nc.scalar.activation(
                        st_all[:, offs[kb] + m0 : offs[kb] + m0 + mw],
                        ps5[:, 0:mw],
                        EXP,
                        scale=SCALE,
                    )
                if kb * 128 >= lo:  # diagonal block: mask in-block triangle
                    nc.vector.tensor_mul(
                        st_all[:, offs[kb] : offs[kb] + 128],
                        st_all[:, offs[kb] : offs[kb] + 128],
                        ut_mask,
                    )

            # AV: per query block j in this group, accumulate over kb<=j
            for j in range(LOB[g], LOB[g] + GBS[g]):
                po = pops.tile([128, 132], F32, tag="po")
                for kb in range(j + 1):
                    s = offs[kb] + j * 128 - max(kb * 128, lo)
                    nc.tensor.matmul(
                        po[:, 0:129],
                        lhsT=st_all[:, s : s + 128],
                        rhs=v_t[kb][:, 0:129],
                        start=(kb == 0),
                        stop=(kb == j),
                    )
                rec = ypool.tile([128, 1], F32, tag="rec")
                nc.vector.reciprocal(rec, po[:, 128:129])
                yb = ypool.tile([128, 128], BF16, tag="yb")
                nc.vector.tensor_scalar_mul(yb, po[:, 0:128], rec)
                ytp = tpsum.tile([128, 4, 128], BF16, tag="tp")
                nc.tensor.transpose(ytp[:, 0, :], yb, ident_b)
                jl = j * 128 - lo
                nc.vector.tensor_copy(yt_g[g][:, h, jl : jl + 128], ytp[:, 0, :])

        # ---- Wo partial rows for this group ----
        partial_g = dram.tile([GBS[g] * 128, C], BF16, tag=f"partial{g}")
        for tb in range(GBS[g]):
            osb = outp.tile([128, C], BF16, tag="osb")
            for no in range(4):
                pw = sps.tile([128, 512], F32, tag="ps5")
                for hh in range(R):
                    nc.tensor.matmul(
                        pw,
                        lhsT=yt_g[g][:, hh, tb * 128 : (tb + 1) * 128],
                        rhs=wo_t[hh][:, no * 512 : (no + 1) * 512],
                        start=(hh == 0),
                        stop=(hh == R - 1),
                    )
                if no % 2 == 0:
                    nc.vector.tensor_copy(osb[:, no * 512 : (no + 1) * 512], pw)
                else:
                    nc.scalar.copy(osb[:, no * 512 : (no + 1) * 512], pw)
            nc.scalar.dma_start(partial_g[tb * 128 : (tb + 1) * 128, :], osb)

        rs_g = dram.tile([GBS[g] * 32, C], BF16, tag=f"rs{g}")
        nc.gpsimd.collective_compute(
            "ReduceScatter",
            mybir.AluOpType.add,
            replica_groups=[[0, 1, 2, 3], [4, 5, 6, 7]],
            ins=[partial_g.opt()],
            outs=[rs_g.opt()],
        )
        rs_tiles.append(rs_g)

        # post-RS for this group right away (gpsimd only, overlaps the
        # remaining groups' compute; only group 3's is true tail)
        off = sum(ROWS_G[:g])
        r0 = 0
        while r0 < ROWS_G[g]:
            rows = min(64, ROWS_G[g] - r0)
            rsb = post.tile([64, C], BF16, tag="rsb")
            nc.gpsimd.dma_start(rsb[0:rows, :], rs_g[r0 : r0 + rows, :])
            osf = post.tile([64, C], F32, tag="osf")
            nc.gpsimd.tensor_copy(osf[0:rows, :], rsb[0:rows, :])
            nc.gpsimd.dma_start(out_d[off + r0 : off + r0 + rows, :], osf[0:rows, :])
            r0 += rows

    for pool in (
        pops, sps, tpsum, projps, dram, post, outp, ypool, stp,
        rope, xtb_pool, xstage, xfpool, wstage, projout, wo_pool, wts, consts,
    ):
        pool.release()


def _shard_inputs(x, cos, sin, Wq, Wkv, Wo):
    in_maps = []
    for c in range(N_CORES):
        b, g = c // KH, c % KH
        in_maps.append(
            {
                "x": np.ascontiguousarray(x[b], dtype=np.float32),
                "cos": np.ascontiguousarray(cos, dtype=np.float32),
                "sin": np.ascontiguousarray(sin, dtype=np.float32),
                "wq": np.ascontiguousarray(
                    Wq[:, g * R * D : (g + 1) * R * D], dtype=np.float32
                ),
                "wk": np.ascontiguousarray(
                    Wkv[:, g * D : (g + 1) * D], dtype=np.float32
                ),
                "wv": np.ascontiguousarray(
                    Wkv[:, KH * D + g * D : KH * D + (g + 1) * D], dtype=np.float32
                ),
                "wo": np.ascontiguousarray(
                    Wo[g * R * D : (g + 1) * R * D, :], dtype=np.float32
                ),
            }
        )
    return in_maps


def get_program():
    if "nc" not in _CACHE:
        _CACHE["nc"] = _build_program()
    return _CACHE["nc"]


def run(x, cos, sin, Wq, Wkv, Wo, **spmd_kwargs):
    nc = get_program()
    in_maps = _shard_inputs(x, cos, sin, Wq, Wkv, Wo)
    res = run_bass_kernel_spmd(
        nc, in_maps, core_ids=list(range(N_CORES)), **spmd_kwargs
    )
    # core (b, r) holds, for each group g, rows
    # [LOB[g]*128 + r*ROWS_G[g], +ROWS_G[g]) of batch b, stored at local
    # offset sum(ROWS_G[:g]).
    out = np.empty((B, T, C), dtype=np.float32)
    for b in range(B):
        for r in range(KH):
            loc = res.results[b * KH + r]["out"]
            off = 0
            for g in range(NG):
                rg = ROWS_G[g]
                g0 = LOB[g] * 128
                out[b, g0 + r * rg : g0 + (r + 1) * rg] = loc[off : off + rg]
                off += rg
    return out, res


def kernel(x, cos, sin, Wq, Wkv, Wo):
    out, _ = run(x, cos, sin, Wq, Wkv, Wo)
    return out
